# revision 1
# baseline (speedup 1.0000x reference)
"""Trainium2 Bass kernel for nn_AttentionRelative (Swin-style relative-position-bias MHA).

Full-problem shapes: x [32, 1024, 512], HEADS=8, DIM_HEAD=64.
Sharding: data-parallel over batch across 8 NeuronCores (4 batches/core);
weights and the (host-gathered, exp'd) bias table replicated.

Device algorithm per (batch, head), all matmul inputs bf16:
  - qkv projection from host-pre-transposed xT, producing qT/kT in
    [head_dim on partitions, n free] layout (one tile per head-pair so
    slots free as the head loop advances) and V in [n on partitions,
    head_dim free] layout with an appended ones-column.
  - S^T = K @ Q^T (keys m on partitions, queries q free) -> PSUM.
  - P^T = exp(S^T) * exp(biasT)  -- ACT does exp straight out of PSUM,
    DVE multiplies by the host-precomputed exp(bias) at bf16 2x rate.
    (exp(s+b) == exp(s)*exp(b); no softmax max-subtraction needed: logits
    are O(+-10) so fp32/bf16 exp is safe.)
  - out^T = [V | 1]^T @ P^T accumulated over m chunks; row 64 of the PSUM
    result is the softmax denominator l[q] for free.
  - l is replicated across partitions via a DRAM bounce (engines cannot
    shift partitions; DMA with a 0-stride DRAM AP can), reciprocal via the
    fast custom DVE op, normalization off the PV critical path.
  - batches are processed in pairs sharing one bias-table load per head
    (halves the dominant DMA stream); the next pair's projections are
    EMITTED inside the current pair's head loop so the scheduler places
    their matmuls into the ACT-bound attention phase's PE bubbles.
  - attnT is repacked to [128, 4, N] (even heads rows 0-63 via direct TT
    writes, odd heads rows 64-127 via partition-shifting SBUF->SBUF DMA),
    which is exactly the lhsT layout the K=128 output projection needs;
    b_out enters as a K=1 ones-row matmul.
"""

import numpy as np
import ml_dtypes

B_FULL = 32
N_CORES = 8
B_LOC = B_FULL // N_CORES  # 4
N = 1024
D = 512
HEADS = 8
DH = 64
NCHUNK = N // 128  # 8
DCHUNK = D // 128  # 4

_PROG = None  # (nc, out_name) built once per process


def _build_program(debug=False):
    import concourse.mybir as mybir
    import concourse.tile as tile
    from concourse import bacc

    f32 = mybir.dt.float32
    bf16 = mybir.dt.bfloat16
    AF = mybir.ActivationFunctionType
    OP = mybir.AluOpType

    nc = bacc.Bacc(target_bir_lowering=False)
    dbg = {}
    if debug:
        dbg["qt"] = nc.dram_tensor("dbg_qt", [128, 4, N], bf16, kind="ExternalOutput")
        dbg["kt"] = nc.dram_tensor("dbg_kt", [128, 4, N], bf16, kind="ExternalOutput")
        dbg["vt"] = nc.dram_tensor(
            "dbg_vt", [128, NCHUNK, HEADS, DH + 1], bf16, kind="ExternalOutput"
        )
        dbg["pt"] = nc.dram_tensor("dbg_pt", [128, N], bf16, kind="ExternalOutput")
        dbg["pv"] = nc.dram_tensor("dbg_pv", [DH + 1, N], f32, kind="ExternalOutput")
        dbg["linvrep"] = nc.dram_tensor(
            "dbg_linvrep", [64, N], f32, kind="ExternalOutput"
        )
        dbg["attn"] = nc.dram_tensor(
            "dbg_attn", [128, DCHUNK, N], bf16, kind="ExternalOutput"
        )

    xt = nc.dram_tensor("xt", [B_LOC, D, N], bf16, kind="ExternalInput")
    wq = nc.dram_tensor("wq", [D, D], bf16, kind="ExternalInput")
    wk = nc.dram_tensor("wk", [D, D], bf16, kind="ExternalInput")
    wv = nc.dram_tensor("wv", [D, D], bf16, kind="ExternalInput")
    wo = nc.dram_tensor("wo", [D, D], bf16, kind="ExternalInput")
    bout = nc.dram_tensor("bout", [1, D], bf16, kind="ExternalInput")
    eb = nc.dram_tensor("eb", [HEADS, N, N], bf16, kind="ExternalInput")
    out = nc.dram_tensor("out", [B_LOC, N, D], f32, kind="ExternalOutput")

    xt_t = xt.rearrange("b (c p) n -> b p c n", p=128)      # [B, 128, 4, N]
    wq_t = wq.rearrange("(c p) m -> p c m", p=128)          # [128, 4, 512]
    wk_t = wk.rearrange("(c p) m -> p c m", p=128)
    wv_t = wv.rearrange("(c p) m -> p c m", p=128)
    wo_t = wo.rearrange("(c p) m -> p c m", p=128)          # [128, 4, 512]
    eb_t = eb.rearrange("h (mc p) q -> h p mc q", p=128)    # [H, 128, 8, N]

    with tile.TileContext(nc) as tc:
        with (
            tc.tile_pool(name="consts", bufs=1) as consts,
            tc.tile_pool(name="xp", bufs=1) as xp,
            tc.tile_pool(name="acts", bufs=3) as acts,
            tc.tile_pool(name="qkp", bufs=12) as qkp,
            tc.tile_pool(name="ep", bufs=2) as ep,
            tc.tile_pool(name="pp", bufs=3) as pp,
            tc.tile_pool(name="attnp", bufs=2) as attnp,
            tc.tile_pool(name="lp", bufs=3) as lp,
            tc.tile_pool(name="outp", bufs=3) as outp,
            tc.tile_pool(name="ldram", bufs=8, space="DRAM") as ldram,
            tc.tile_pool(name="ps_proj", bufs=2, space="PSUM") as ps_proj,
            tc.tile_pool(name="ps_s", bufs=2, space="PSUM") as ps_s,
            tc.tile_pool(name="ps_pv", bufs=2, space="PSUM") as ps_pv,
        ):
            wq_sb = consts.tile([128, DCHUNK, D], bf16, tag="wq")
            wk_sb = consts.tile([128, DCHUNK, D], bf16, tag="wk")
            wv_sb = consts.tile([128, DCHUNK, D], bf16, tag="wv")
            wo_sb = consts.tile([128, DCHUNK, D], bf16, tag="wo")
            bout_sb = consts.tile([1, D], bf16, tag="bout")
            ones_sb = consts.tile([1, 128], bf16, tag="ones1")
            nc.sync.dma_start(wq_sb, wq_t)
            nc.sync.dma_start(wk_sb, wk_t)
            nc.sync.dma_start(wv_sb, wv_t)
            nc.sync.dma_start(wo_sb, wo_t)
            nc.sync.dma_start(bout_sb, bout[:, :])
            nc.gpsimd.memset(ones_sb, 1.0)

            tiles = {}  # b -> (qt_p, kt_p, vt_sb, attn2)
            xts = {}

            def emit_qk(b, pairs=range(4)):
                if b not in xts:
                    xt_sb = xp.tile([128, DCHUNK, N], bf16, tag="xt", name=f"xt_{b}")
                    for dc in range(DCHUNK):
                        nc.sync.dma_start(xt_sb[:, dc, :], xt_t[b, :, dc, :])
                    xts[b] = xt_sb
                    tiles[b] = [
                        [
                            qkp.tile([128, N], bf16, tag="qt_pair", name=f"qt_{b}_{p}")
                            for p in range(4)
                        ],
                        [
                            qkp.tile([128, N], bf16, tag="kt_pair", name=f"kt_{b}_{p}")
                            for p in range(4)
                        ],
                        None,
                        None,
                    ]
                xt_sb = xts[b]
                qt_p, kt_p = tiles[b][0], tiles[b][1]
                # Q/K projections: two heads (128 cols) per matmul group,
                # one tile per head-pair so slots free as heads complete.
                for p in pairs:
                    for w_sb, dstl in ((wq_sb, qt_p), (wk_sb, kt_p)):
                        for half in range(2):
                            ps = ps_proj.tile([128, 512], f32, tag="ps_proj")
                            for dc in range(DCHUNK):
                                nc.tensor.matmul(
                                    ps,
                                    lhsT=w_sb[:, dc, p * 128 : (p + 1) * 128],
                                    rhs=xt_sb[:, dc, half * 512 : (half + 1) * 512],
                                    start=(dc == 0),
                                    stop=(dc == DCHUNK - 1),
                                )
                            nc.vector.tensor_copy(
                                dstl[p][:, half * 512 : (half + 1) * 512], ps
                            )

                if debug and b == 0 and list(pairs)[-1] == 3:
                    for p in range(4):
                        nc.sync.dma_start(dbg["qt"][:, p, :], qt_p[p])
                        nc.sync.dma_start(dbg["kt"][:, p, :], kt_p[p])

            def emit_v(b):
                xt_sb = xts[b]
                vt_sb = acts.tile(
                    [128, NCHUNK, HEADS, DH + 1], bf16, tag="vt", name=f"vt_{b}"
                )
                nc.gpsimd.memset(vt_sb[:, :, :, DH : DH + 1], 1.0)
                # V projection: natural [n, inner] layout.
                for nck in range(NCHUNK):
                    ps = ps_proj.tile([128, 512], f32, tag="ps_proj")
                    for dc in range(DCHUNK):
                        nc.tensor.matmul(
                            ps,
                            lhsT=xt_sb[:, dc, nck * 128 : (nck + 1) * 128],
                            rhs=wv_sb[:, dc, :],
                            start=(dc == 0),
                            stop=(dc == DCHUNK - 1),
                        )
                    nc.vector.tensor_copy(
                        vt_sb[:, nck, :, 0:DH],
                        ps.rearrange("p (h d) -> p h d", h=HEADS),
                    )

                if debug and b == 0:
                    nc.sync.dma_start(dbg["vt"][:, :, :, :], vt_sb)

                attn2 = attnp.tile(
                    [128, DCHUNK, N], bf16, tag="attn2", name=f"attn2_{b}"
                )
                tiles[b][2] = vt_sb
                tiles[b][3] = attn2

            def emit_qkv(b):
                emit_qk(b)
                emit_v(b)

            pending_epi = []  # deferred recip+normalize closures

            def emit_head(bpair, h):
                po = (h % 2) * 64
                pr = h // 2
                eb_sb = ep.tile([128, NCHUNK, N], bf16, tag="eb", name=f"eb_{bpair[0]}_{h}")
                for mc in range(NCHUNK):
                    nc.sync.dma_start(eb_sb[:, mc, :], eb_t[h, :, mc, :])

                for b in bpair:
                    qt_p, kt_p, vt_sb, attn2 = tiles[b]
                    pv0 = ps_pv.tile([DH + 1, 512], f32, tag="pv", name=f"pv0_{b}_{h}")
                    pv1 = ps_pv.tile([DH + 1, 512], f32, tag="pv", name=f"pv1_{b}_{h}")

                    for mc in range(NCHUNK):
                        ps = ps_s.tile([128, N], f32, tag="ps_s")
                        for half in range(2):
                            nc.tensor.matmul(
                                ps[:, half * 512 : (half + 1) * 512],
                                lhsT=kt_p[pr][po : po + 64, mc * 128 : (mc + 1) * 128],
                                rhs=qt_p[pr][po : po + 64, half * 512 : (half + 1) * 512],
                                start=True,
                                stop=True,
                            )
                        p_raw = pp.tile([128, N], bf16, tag="praw")
                        nc.scalar.activation(p_raw, ps, AF.Exp)
                        p_t = pp.tile([128, N], bf16, tag="pt")
                        nc.vector.tensor_tensor(p_t, p_raw, eb_sb[:, mc, :], OP.mult)
                        if debug and b == 0 and h == 0 and mc == 0:
                            nc.sync.dma_start(dbg["pt"][:, :], p_t)
                        # Inject the previous iteration's deferred epilogue
                        # mid-loop: its reciprocal waits on the l DRAM-bounce
                        # DMA, and DVE executes its stream in order, so placed
                        # here the wait overlaps this iteration's multiplies
                        # instead of stalling them.
                        if mc == 3 and pending_epi:
                            pending_epi.pop(0)()
                        for half, pv in ((0, pv0), (1, pv1)):
                            nc.tensor.matmul(
                                pv,
                                lhsT=vt_sb[:, mc, h, :],
                                rhs=p_t[:, half * 512 : (half + 1) * 512],
                                start=(mc == 0),
                                stop=(mc == NCHUNK - 1),
                            )

                    # Evacuate o_T (rows 0-63) and l (row 64) in one f32
                    # copy per half; pv psum frees immediately after. The
                    # DRAM-bounce DMAs are issued now; recip+normalize are
                    # deferred into the next iteration's emission.
                    au = pp.tile([DH + 1, N], f32, tag="au")
                    nc.vector.tensor_copy(au[:, 0:512], pv0)
                    nc.scalar.activation(au[:, 512:1024], pv1, AF.Copy)
                    l_dram = ldram.tile([1, N], f32, tag="l_dram")
                    nc.sync.dma_start(l_dram, au[DH : DH + 1, :])
                    l_rep = lp.tile([64, N], f32, tag="l_rep")
                    nc.sync.dma_start(l_rep, l_dram.to_broadcast((64, N)))

                    def epi(au=au, l_rep=l_rep, attn2=attn2, pr=pr, h=h, b=b):
                        linv_rep = lp.tile([64, N], f32, tag="linv_rep")
                        nc.vector.reciprocal_approx_fast(out=linv_rep, in_=l_rep)
                        if debug and b == 0 and h == 0:
                            nc.sync.dma_start(dbg["pv"][:, :], au)
                            nc.sync.dma_start(dbg["linvrep"][:, :], linv_rep)
                        if h % 2 == 0:
                            nc.vector.tensor_tensor(
                                attn2[0:64, pr, :], au[0:DH, :], linv_rep, OP.mult
                            )
                        else:
                            attn_odd = pp.tile([64, N], bf16, tag="attn_odd")
                            nc.vector.tensor_tensor(
                                attn_odd, au[0:DH, :], linv_rep, OP.mult
                            )
                            nc.sync.dma_start(attn2[64:128, pr, :], attn_odd)

                    pending_epi.append(epi)

            def emit_proj(b):
                attn2 = tiles[b][3]
                if debug and b == 0:
                    nc.sync.dma_start(dbg["attn"][:, :, :], attn2)
                for nck in range(NCHUNK):
                    ps = ps_proj.tile([128, 512], f32, tag="ps_proj")
                    for ic in range(DCHUNK):
                        nc.tensor.matmul(
                            ps,
                            lhsT=attn2[:, ic, nck * 128 : (nck + 1) * 128],
                            rhs=wo_sb[:, ic, :],
                            start=(ic == 0),
                            stop=False,
                        )
                    nc.tensor.matmul(
                        ps,
                        lhsT=ones_sb[0:1, :],
                        rhs=bout_sb[0:1, :],
                        start=False,
                        stop=True,
                    )
                    o_sb = outp.tile([128, D], f32, tag="osb")
                    nc.scalar.activation(o_sb, ps, AF.Copy)
                    nc.sync.dma_start(out[b, nck * 128 : (nck + 1) * 128, :], o_sb)

            # software-pipelined emission: the next pair's projections are
            # emitted inside the current pair's head loop so their matmuls
            # can fill PE bubbles of the ACT-bound attention phase.
            emit_qkv(0)
            emit_qkv(1)
            pending_proj = []
            for bp in range(B_LOC // 2):
                bpair = (2 * bp, 2 * bp + 1)
                nxt = (2 * bp + 2, 2 * bp + 3) if bp + 1 < B_LOC // 2 else None
                for h in range(HEADS):
                    emit_head(bpair, h)
                    if pending_proj and h == 1:
                        emit_proj(pending_proj.pop(0))
                    elif pending_proj and h == 4:
                        emit_proj(pending_proj.pop(0))
                    if nxt is not None:
                        if h == 2:
                            emit_qk(nxt[0])
                        elif h == 3:
                            emit_v(nxt[0])
                        elif h == 5:
                            emit_qk(nxt[1])
                        elif h == 6:
                            emit_v(nxt[1])
                pending_proj += [bpair[0], bpair[1]]
            for e in pending_epi:
                e()
            pending_epi.clear()
            for b in pending_proj:
                emit_proj(b)

    nc.compile()
    return nc, "out"


def get_program():
    global _PROG
    if _PROG is None:
        _PROG = _build_program()
    return _PROG


def shard_inputs(x, w_qkv, w_out, b_out, bias_table, rel_index):
    bf = ml_dtypes.bfloat16
    x = np.asarray(x, np.float32)
    w_qkv = np.asarray(w_qkv, np.float32)
    w_out = np.asarray(w_out, np.float32)
    b_out = np.asarray(b_out, np.float32)
    bias_table = np.asarray(bias_table, np.float32)
    rel_index = np.asarray(rel_index)

    scale = DH ** -0.5
    wq = np.ascontiguousarray(w_qkv[:, 0:D] * scale).astype(bf)
    wk = np.ascontiguousarray(w_qkv[:, D : 2 * D]).astype(bf)
    wv = np.ascontiguousarray(w_qkv[:, 2 * D : 3 * D]).astype(bf)
    wo = np.ascontiguousarray(w_out).astype(bf)
    bout = np.ascontiguousarray(b_out[None, :]).astype(bf)
    # exp(bias)^T per head: eb[h, m, q] = exp(bias_table[rel_index[q, m], h])
    biasT = bias_table[rel_index].transpose(2, 1, 0)  # [H, m, q]
    ebv = np.ascontiguousarray(np.exp(biasT).astype(bf))

    in_maps = []
    for c in range(N_CORES):
        xs = x[c * B_LOC : (c + 1) * B_LOC]  # [4, 1024, 512]
        xts = np.ascontiguousarray(xs.transpose(0, 2, 1)).astype(bf)
        in_maps.append(
            {
                "xt": xts,
                "wq": wq,
                "wk": wk,
                "wv": wv,
                "wo": wo,
                "bout": bout,
                "eb": ebv,
            }
        )
    return in_maps


def kernel(x, w_qkv, w_out, b_out, bias_table, rel_index):
    from concourse.bass_utils import run_bass_kernel_spmd

    nc, out_name = get_program()
    in_maps = shard_inputs(x, w_qkv, w_out, b_out, bias_table, rel_index)
    try:
        res = run_bass_kernel_spmd(nc, in_maps, core_ids=list(range(N_CORES)))
    except Exception:
        # transient device errors (e.g. NRT_EXEC_UNIT_UNRECOVERABLE) have been
        # observed once on an otherwise-passing kernel; retry once
        res = run_bass_kernel_spmd(nc, in_maps, core_ids=list(range(N_CORES)))
    outs = [r[out_name] for r in res.results]
    return np.concatenate(outs, axis=0).astype(np.float32)

